# revision 9
# baseline (speedup 1.0000x reference)
"""Trainium2 Bass kernel: 16-head RoPE attention (B=2, L=2048, HIDDEN=1024).

Sharding: 8 cores = 2 batches x 4 head-groups (4 heads per core).
v2: all matmul inputs bf16 (same PE rate as fp32r, half the SBUF/DMA),
x resident in SBUF (loaded once), scalar queue runs exp only, o-proj
DMAs psum->DRAM directly, short normalization chain (reciprocal on one
partition + partition_broadcast), k-proj-first prologue so exp starts
early.
"""

import numpy as np
import ml_dtypes
from contextlib import ExitStack

from concourse import bacc, tile, mybir
from concourse.bass import ts
from concourse.bass_utils import run_bass_kernel_spmd

HIDDEN = 1024
HEADS = 16
HD = 64
L = 2048
B = 2
BASE = 10000.0

P = 128
E_LOCAL = 256          # 4 heads per core
N_PAIRS = 2            # head pairs per core (2 heads on 128 partitions)
HC = HIDDEN // P       # 8 hidden chunks
TC = 512               # token chunk (matmul free dim)
N_TC = L // TC         # 4
N_TT = L // P          # 16 token tiles (for v / k-tiles)
SCALE = 1.0 / 8.0      # 1/sqrt(HD)

F32 = mybir.dt.float32
BF16 = mybir.dt.bfloat16
AF = mybir.ActivationFunctionType


def build_program(debug=False):
    nc = bacc.Bacc(None, target_bir_lowering=False)
    names = {}
    with tile.TileContext(nc) as tc:
        ctx = ExitStack()
        with ctx:
            dram = ctx.enter_context(tc.tile_pool(name="dram", bufs=1, space="DRAM"))
            xT_d = dram.tile([HIDDEN, L], BF16, kind="ExternalInput", name="xT")
            wq_d = dram.tile([HIDDEN, E_LOCAL], BF16, kind="ExternalInput", name="wq")
            wk_d = dram.tile([HIDDEN, E_LOCAL], BF16, kind="ExternalInput", name="wk")
            wv_d = dram.tile([HIDDEN, E_LOCAL], BF16, kind="ExternalInput", name="wv")
            wo_d = dram.tile([E_LOCAL, HIDDEN], BF16, kind="ExternalInput", name="wo")
            cos_d = dram.tile([P, L], BF16, kind="ExternalInput", name="cosT")
            sin_d = dram.tile([P, L], BF16, kind="ExternalInput", name="sinT")
            out_d = dram.tile([HIDDEN, L], F32, kind="ExternalOutput", name="outT")
            names["in"] = [t.tensor.name for t in (xT_d, wq_d, wk_d, wv_d, wo_d, cos_d, sin_d)]
            names["out"] = out_d.tensor.name

            # ---------------- persistent SBUF ----------------
            const = ctx.enter_context(tc.tile_pool(name="const", bufs=1))
            wq_sb = const.tile([P, HC, E_LOCAL], BF16)
            wk_sb = const.tile([P, HC, E_LOCAL], BF16)
            wv_sb = const.tile([P, HC, E_LOCAL], BF16)
            wo_sb = const.tile([P, 2, HIDDEN], BF16)
            cos_sb = const.tile([P, L], BF16)
            sin_sb = const.tile([P, L], BF16)
            xsb = const.tile([P, HC, L], BF16)

            # rope'd q and k, feature-major: per pair [128, L] bf16
            qkro = ctx.enter_context(tc.tile_pool(name="qkro", bufs=1))
            q_ro = [qkro.tile([P, L], BF16, name=f"q_ro{p}") for p in range(N_PAIRS)]
            k_ro = [qkro.tile([P, L], BF16, name=f"k_ro{p}") for p in range(N_PAIRS)]
            # v token-major with ones columns: [128 tok, tt, 4*65] bf16
            v_all = qkro.tile([P, N_TT, 4 * (HD + 1)], BF16)
            ones_sb = qkro.tile([P, N_TT], BF16)
            nc.vector.memset(ones_sb[:], 1.0)
            for g in range(4):
                nc.vector.tensor_copy(
                    v_all[:, :, g * (HD + 1) + HD : g * (HD + 1) + HD + 1],
                    ones_sb[:].rearrange("p (a b) -> p a b", b=1),
                )
            # normalized attention output, feature-major per pair [128, L] bf16
            o_sb = [qkro.tile([P, L], BF16, name=f"o_sb{p}") for p in range(N_PAIRS)]

            # ---------------- working pools ----------------
            rope_t = ctx.enter_context(tc.tile_pool(name="rope", bufs=2))
            expp = ctx.enter_context(tc.tile_pool(name="expp", bufs=3))
            nrm = ctx.enter_context(tc.tile_pool(name="nrm", bufs=2))
            outst = ctx.enter_context(tc.tile_pool(name="outst", bufs=2))

            def rope_chunk(dst, ps_tile, t):
                """psum [128, TC] -> dst[:, t*TC:(t+1)*TC] with RoPE applied."""
                raw = rope_t.tile([P, TC], F32, name="raw")
                shuf = rope_t.tile([P, TC], F32, name="shuf")
                t1 = rope_t.tile([P, TC], F32, name="t1")
                t2 = rope_t.tile([P, TC], F32, name="t2")
                # fast psum release: one copy out, everything else reads raw
                nc.vector.tensor_copy(raw[:], ps_tile[:])
                # swap 32-partition halves within each 64-row head block
                nc.gpsimd.dma_start(shuf[0:32, :], raw[32:64, :])
                nc.gpsimd.dma_start(shuf[32:64, :], raw[0:32, :])
                nc.sync.dma_start(shuf[64:96, :], raw[96:128, :])
                nc.sync.dma_start(shuf[96:128, :], raw[64:96, :])
                nc.vector.tensor_mul(t1[:], raw[:], cos_sb[:, ts(t, TC)])
                nc.vector.tensor_mul(t2[:], shuf[:], sin_sb[:, ts(t, TC)])
                nc.vector.tensor_add(dst[:, ts(t, TC)], t1[:], t2[:])

            def proj_chunk(dst_ro, w_sb, pair, t, tag, ps_qk):
                """One [128, TC] projection chunk for q or k + rope."""
                pp = ps_qk.tile([P, TC], F32, name=tag, tag=tag, bufs=1)
                for h in range(HC):
                    nc.tensor.matmul(
                        pp[:], w_sb[:, h, ts(pair, P)], xsb[:, h, ts(t, TC)],
                        start=(h == 0), stop=(h == HC - 1),
                    )
                rope_chunk(dst_ro, pp, t)

            def v_tiles(tt_list, ps_qk):
                for tt in tt_list:
                    vp = ps_qk.tile(
                        [P, E_LOCAL], F32, name="vp",
                        tag=("qp" if tt % 2 == 0 else "kp"), bufs=1,
                    )
                    for h in range(HC):
                        nc.tensor.matmul(
                            vp[:], xsb[:, h, ts(tt, P)], wv_sb[:, h, :],
                            start=(h == 0), stop=(h == HC - 1),
                        )
                    # single copy: [128, 4 groups, 64] strided into 65-wide slots
                    nc.vector.tensor_copy(
                        v_all[:, tt, :].rearrange("p (g c) -> p g c", g=4)[:, :, 0:HD],
                        vp[:].rearrange("p (g c) -> p g c", g=4),
                    )

            def attention_chunk(pair, c, ps_st, ps_ot, mid_filler=None):
                ot = ps_ot.tile([HD + 1, 2 * TC], F32, name="ot", bufs=1)
                for kt in range(N_TT):
                    if kt == 8 and mid_filler is not None:
                        mid_filler()
                    st = ps_st.tile([P, 2 * TC], F32, name="st", tag="st")
                    nc.tensor.matmul(
                        st[:, 0:TC],
                        k_ro[pair][0:HD, ts(kt, P)],
                        q_ro[pair][0:HD, ts(c, TC)],
                        start=True, stop=True,
                    )
                    nc.tensor.matmul(
                        st[:, TC : 2 * TC],
                        k_ro[pair][HD:P, ts(kt, P)],
                        q_ro[pair][HD:P, ts(c, TC)],
                        start=True, stop=True,
                        tile_position=(64, 0),
                    )
                    ex = expp.tile([P, 2 * TC], BF16, name="ex")
                    nc.scalar.activation(ex[:], st[:], AF.Exp, scale=SCALE)
                    for hd_i in range(2):
                        g = 2 * pair + hd_i
                        nc.tensor.matmul(
                            ot[:, ts(hd_i, TC)],
                            v_all[:, kt, g * (HD + 1) : (g + 1) * (HD + 1)],
                            ex[:, ts(hd_i, TC)],
                            start=(kt == 0), stop=(kt == N_TT - 1),
                        )
                # normalization: copy out psum fast, reciprocal on the sum row,
                # broadcast, two muls; h2 half moves partitions via one DMA
                n65 = nrm.tile([HD + 1, 2 * TC], F32, name="n65")
                nc.vector.tensor_copy(n65[:], ot[:])
                inv = nrm.tile([1, 2 * TC], F32, name="inv")
                nc.vector.reciprocal(inv[:], n65[HD : HD + 1, :])
                bsum = nrm.tile([HD, 2 * TC], F32, name="bsum")
                nc.gpsimd.partition_broadcast(bsum[:], inv[:])
                nc.vector.tensor_mul(
                    o_sb[pair][0:HD, ts(c, TC)], n65[0:HD, 0:TC], bsum[:, 0:TC]
                )
                onrm = nrm.tile([HD, TC], BF16, name="onrm")
                nc.vector.tensor_mul(onrm[:], n65[0:HD, TC : 2 * TC], bsum[:, TC : 2 * TC])
                nc.gpsimd.dma_start(o_sb[pair][HD:P, ts(c, TC)], onrm[:])

            def o_proj_chunk(t, ps_qk, fcs=range(HC)):
                for fc in fcs:
                    op = ps_qk.tile(
                        [P, TC], F32, name="op", tag=("qp" if fc % 2 == 0 else "kp"), bufs=1
                    )
                    for pair in range(N_PAIRS):
                        nc.tensor.matmul(
                            op[:],
                            wo_sb[:, pair, ts(fc, P)],
                            o_sb[pair][:, ts(t, TC)],
                            start=(pair == 0), stop=(pair == N_PAIRS - 1),
                        )
                    # bounce via SBUF (DMA cannot read PSUM); keep scalar free
                    ob = outst.tile([P, TC], F32, name="ob")
                    nc.vector.tensor_copy(ob[:], op[:])
                    if t == N_TC - 1:
                        # scalar queue is free after the last exp
                        out_eng = (nc.sync, nc.gpsimd, nc.scalar)[fc % 3]
                    else:
                        out_eng = (nc.sync, nc.gpsimd)[fc % 2]
                    out_eng.dma_start(out_d[ts(fc, P), ts(t, TC)], ob[:])

            # ---- emission order drives scheduler priority ----
            with tc.tile_pool(name="ps_qk", bufs=1, space="PSUM") as ps_qk:
                with tc.tile_pool(name="ps_st", bufs=2, space="PSUM") as ps_st:
                    with tc.tile_pool(name="ps_ot", bufs=1, space="PSUM") as ps_ot:
                        # input DMAs: x quarters split over two queues, ahead
                        # of cos/sin which are only needed once rope starts
                        nc.sync.dma_start(wk_sb[:], wk_d[:].rearrange("(c p) e -> p c e", p=P))
                        nc.scalar.dma_start(
                            xsb[:, :, ts(0, TC)],
                            xT_d[:, ts(0, TC)].rearrange("(c p) t -> p c t", p=P),
                        )
                        nc.sync.dma_start(
                            xsb[:, :, ts(1, TC)],
                            xT_d[:, ts(1, TC)].rearrange("(c p) t -> p c t", p=P),
                        )
                        nc.gpsimd.dma_start(cos_sb[:], cos_d[:])
                        nc.gpsimd.dma_start(sin_sb[:], sin_d[:])
                        nc.scalar.dma_start(
                            xsb[:, :, ts(2, TC)],
                            xT_d[:, ts(2, TC)].rearrange("(c p) t -> p c t", p=P),
                        )
                        nc.sync.dma_start(
                            xsb[:, :, ts(3, TC)],
                            xT_d[:, ts(3, TC)].rearrange("(c p) t -> p c t", p=P),
                        )
                        nc.sync.dma_start(wq_sb[:], wq_d[:].rearrange("(c p) e -> p c e", p=P))
                        nc.gpsimd.dma_start(wv_sb[:], wv_d[:].rearrange("(c p) e -> p c e", p=P))
                        nc.gpsimd.dma_start(wo_sb[:], wo_d[:].rearrange("(c p) f -> p c f", p=P))

                        # prologue: k first (scores need all of k), then q c0 + v,
                        # so exp can start as early as possible
                        for t in range(N_TC):
                            proj_chunk(k_ro[0], wk_sb, 0, t, "qp" if t % 2 == 0 else "kp", ps_qk)
                        proj_chunk(q_ro[0], wq_sb, 0, 0, "qp", ps_qk)
                        v_tiles(range(0, 4), ps_qk)
                        for t in range(1, N_TC):
                            proj_chunk(q_ro[0], wq_sb, 0, t, "qp" if t % 2 == 0 else "kp", ps_qk)
                        v_tiles(range(4, N_TT), ps_qk)

                        # attention pair 0 with pair-1 projections as PE filler,
                        # one chunk mid-attention and one at the chunk boundary so
                        # psum-bank reuse chains get ~9us of slack
                        p1 = [
                            (k_ro[1], wk_sb), (k_ro[1], wk_sb), (k_ro[1], wk_sb),
                            (k_ro[1], wk_sb), (q_ro[1], wq_sb), (q_ro[1], wq_sb),
                            (q_ro[1], wq_sb), (q_ro[1], wq_sb),
                        ]
                        p1_t = [0, 1, 2, 3, 0, 1, 2, 3]
                        fi = iter(range(8))

                        def filler1():
                            i = next(fi)
                            dst, w = p1[i]
                            proj_chunk(dst, w, 1, p1_t[i], "qp" if i % 2 == 0 else "kp", ps_qk)

                        for c in range(N_TC):
                            attention_chunk(0, c, ps_st, ps_ot, mid_filler=filler1)
                            filler1()

                        # attention pair 1 with o-proj halves as staggered filler
                        for c in range(N_TC):
                            if c >= 1:
                                mid = lambda t=c - 1: o_proj_chunk(t, ps_qk, fcs=range(0, 4))
                            else:
                                mid = None
                            attention_chunk(1, c, ps_st, ps_ot, mid_filler=mid)
                            if c >= 1:
                                o_proj_chunk(c - 1, ps_qk, fcs=range(4, HC))
                        o_proj_chunk(N_TC - 1, ps_qk)

    nc.compile()
    return nc, names


_CACHE = {}


def _get_program():
    if "prog" not in _CACHE:
        _CACHE["prog"] = build_program()
    return _CACHE["prog"]


def _rope_tables():
    inv_freq = 1.0 / (BASE ** (np.arange(0, HD, 2, dtype=np.float64) / HD))
    t = np.arange(L, dtype=np.float64)
    freqs = np.outer(t, inv_freq)            # [L, 32]
    emb = np.concatenate((freqs, freqs), -1)  # [L, 64]
    cos = np.cos(emb).T.astype(np.float32)    # [64, L]
    sin = np.sin(emb).T.astype(np.float32)    # [64, L]
    sin_signed = sin.copy()
    sin_signed[: HD // 2] *= -1.0             # rotate_half sign baked in
    cosT = np.ascontiguousarray(np.concatenate([cos, cos], 0))      # [128, L]
    sinT = np.ascontiguousarray(np.concatenate([sin_signed, sin_signed], 0))
    return cosT.astype(ml_dtypes.bfloat16), sinT.astype(ml_dtypes.bfloat16)


def make_in_maps(names, x, Wq, Wk, Wv, Wo):
    cosT, sinT = _rope_tables()
    bf = ml_dtypes.bfloat16
    in_maps = []
    xTs = [np.ascontiguousarray(x[b].T).astype(bf) for b in range(B)]
    for core in range(8):
        b = core // 4
        g = core % 4
        es = slice(g * E_LOCAL, (g + 1) * E_LOCAL)
        m = {
            names["in"][0]: xTs[b],
            names["in"][1]: np.ascontiguousarray(Wq[es, :].T).astype(bf),  # [1024, 256]
            names["in"][2]: np.ascontiguousarray(Wk[es, :].T).astype(bf),
            names["in"][3]: np.ascontiguousarray(Wv[es, :].T).astype(bf),
            names["in"][4]: np.ascontiguousarray(Wo[:, es].T).astype(bf),  # [256, 1024]
            names["in"][5]: cosT,
            names["in"][6]: sinT,
        }
        in_maps.append(m)
    return in_maps


def gather_out(names, res):
    out = np.zeros((B, L, HIDDEN), dtype=np.float32)
    for b in range(B):
        acc = np.zeros((HIDDEN, L), dtype=np.float32)
        for g in range(4):
            acc += np.asarray(res.results[b * 4 + g][names["out"]], dtype=np.float32)
        out[b] = acc.T
    return out


def kernel(x, Wq, Wk, Wv, Wo):
    x = np.asarray(x, dtype=np.float32)
    Wq = np.asarray(Wq, dtype=np.float32)
    Wk = np.asarray(Wk, dtype=np.float32)
    Wv = np.asarray(Wv, dtype=np.float32)
    Wo = np.asarray(Wo, dtype=np.float32)

    nc, names = _get_program()
    in_maps = make_in_maps(names, x, Wq, Wk, Wv, Wo)
    res = run_bass_kernel_spmd(nc, in_maps, core_ids=list(range(8)))
    return gather_out(names, res)


# revision 18
# speedup vs baseline: 1.0292x; 1.0292x over previous
"""Trainium2 Bass kernel: 16-head RoPE attention (B=2, L=2048, HIDDEN=1024).

Sharding: 8 cores = 2 batches x 4 head-groups (4 heads per core).
v2: all matmul inputs bf16 (same PE rate as fp32r, half the SBUF/DMA),
x resident in SBUF (loaded once), scalar queue runs exp only, o-proj
DMAs psum->DRAM directly, short normalization chain (reciprocal on one
partition + partition_broadcast), k-proj-first prologue so exp starts
early.
"""

import numpy as np
import ml_dtypes
from contextlib import ExitStack

from concourse import bacc, tile, mybir
from concourse.bass import ts
from concourse.bass_utils import run_bass_kernel_spmd

HIDDEN = 1024
HEADS = 16
HD = 64
L = 2048
B = 2
BASE = 10000.0

P = 128
E_LOCAL = 256          # 4 heads per core
N_PAIRS = 2            # head pairs per core (2 heads on 128 partitions)
HC = HIDDEN // P       # 8 hidden chunks
TC = 512               # token chunk (matmul free dim)
N_TC = L // TC         # 4
N_TT = L // P          # 16 token tiles (for v / k-tiles)
SCALE = 1.0 / 8.0      # 1/sqrt(HD)

F32 = mybir.dt.float32
BF16 = mybir.dt.bfloat16
AF = mybir.ActivationFunctionType


def build_program(debug=False):
    nc = bacc.Bacc(None, target_bir_lowering=False)
    names = {}
    with tile.TileContext(nc) as tc:
        ctx = ExitStack()
        with ctx:
            dram = ctx.enter_context(tc.tile_pool(name="dram", bufs=1, space="DRAM"))
            # host pre-arranges everything partition-major so each DMA is 128
            # large contiguous descriptors (fast DGE) instead of ~1024 small ones
            xT_d = dram.tile([P, N_TC, HC, TC], BF16, kind="ExternalInput", name="xT")
            wq_d = dram.tile([P, HC, E_LOCAL], BF16, kind="ExternalInput", name="wq")
            wk_d = dram.tile([P, HC, E_LOCAL], BF16, kind="ExternalInput", name="wk")
            wv_d = dram.tile([P, HC, E_LOCAL], BF16, kind="ExternalInput", name="wv")
            wo_d = dram.tile([P, 2, HIDDEN], BF16, kind="ExternalInput", name="wo")
            cos_d = dram.tile([P, L], BF16, kind="ExternalInput", name="cosT")
            sin_d = dram.tile([P, L], BF16, kind="ExternalInput", name="sinT")
            out_d = dram.tile([HIDDEN, L], F32, kind="ExternalOutput", name="outT")
            names["in"] = [t.tensor.name for t in (xT_d, wq_d, wk_d, wv_d, wo_d, cos_d, sin_d)]
            names["out"] = out_d.tensor.name

            # ---------------- persistent SBUF ----------------
            const = ctx.enter_context(tc.tile_pool(name="const", bufs=1))
            wq_sb = const.tile([P, HC, E_LOCAL], BF16)
            wk_sb = const.tile([P, HC, E_LOCAL], BF16)
            wv_sb = const.tile([P, HC, E_LOCAL], BF16)
            wo_sb = const.tile([P, 2, HIDDEN], BF16)
            cos_sb = const.tile([P, L], BF16)
            sin_sb = const.tile([P, L], BF16)
            xsb = const.tile([P, N_TC, HC, TC], BF16)

            # rope'd q and k, feature-major: per pair [128, L] bf16
            qkro = ctx.enter_context(tc.tile_pool(name="qkro", bufs=1))
            q_ro = [qkro.tile([P, L], BF16, name=f"q_ro{p}") for p in range(N_PAIRS)]
            k_ro = [qkro.tile([P, L], BF16, name=f"k_ro{p}") for p in range(N_PAIRS)]
            # v token-major with ones columns: [128 tok, tt, 4*65] bf16
            v_all = qkro.tile([P, N_TT, 4 * (HD + 1)], BF16)
            ones_sb = qkro.tile([P, N_TT], BF16)
            nc.vector.memset(ones_sb[:], 1.0)
            for g in range(4):
                nc.vector.tensor_copy(
                    v_all[:, :, g * (HD + 1) + HD : g * (HD + 1) + HD + 1],
                    ones_sb[:].rearrange("p (a b) -> p a b", b=1),
                )
            # normalized attention output, feature-major per pair [128, L] bf16
            o_sb = [qkro.tile([P, L], BF16, name=f"o_sb{p}") for p in range(N_PAIRS)]

            # ---------------- working pools ----------------
            rope_t = ctx.enter_context(tc.tile_pool(name="rope", bufs=2))
            expp = ctx.enter_context(tc.tile_pool(name="expp", bufs=3))
            nrm = ctx.enter_context(tc.tile_pool(name="nrm", bufs=2))
            outst = ctx.enter_context(tc.tile_pool(name="outst", bufs=2))

            def rope_chunk(dst, ps_tile, t):
                """psum [128, TC] -> dst[:, t*TC:(t+1)*TC] with RoPE applied."""
                raw = rope_t.tile([P, TC], F32, name="raw")
                shuf = rope_t.tile([P, TC], F32, name="shuf")
                t1 = rope_t.tile([P, TC], F32, name="t1")
                t2 = rope_t.tile([P, TC], F32, name="t2")
                # fast psum release: one copy out, everything else reads raw
                nc.vector.tensor_copy(raw[:], ps_tile[:])
                # swap 32-partition halves within each 64-row head block
                nc.gpsimd.dma_start(shuf[0:32, :], raw[32:64, :])
                nc.gpsimd.dma_start(shuf[32:64, :], raw[0:32, :])
                nc.sync.dma_start(shuf[64:96, :], raw[96:128, :])
                nc.sync.dma_start(shuf[96:128, :], raw[64:96, :])
                nc.vector.tensor_mul(t1[:], raw[:], cos_sb[:, ts(t, TC)])
                nc.vector.tensor_mul(t2[:], shuf[:], sin_sb[:, ts(t, TC)])
                nc.vector.tensor_add(dst[:, ts(t, TC)], t1[:], t2[:])

            def proj_chunk(dst_ro, w_sb, pair, t, tag, ps_qk):
                """One [128, TC] projection chunk for q or k + rope."""
                pp = ps_qk.tile([P, TC], F32, name=tag, tag=tag, bufs=1)
                for h in range(HC):
                    nc.tensor.matmul(
                        pp[:], w_sb[:, h, ts(pair, P)], xsb[:, t, h, :],
                        start=(h == 0), stop=(h == HC - 1),
                    )
                rope_chunk(dst_ro, pp, t)

            def v_tiles(tt_list, ps_qk):
                for tt in tt_list:
                    q, s = tt // 4, tt % 4
                    vp = ps_qk.tile(
                        [P, E_LOCAL], F32, name="vp",
                        tag=("qp" if tt % 2 == 0 else "kp"), bufs=1,
                    )
                    for h in range(HC):
                        nc.tensor.matmul(
                            vp[:], xsb[:, q, h, ts(s, P)], wv_sb[:, h, :],
                            start=(h == 0), stop=(h == HC - 1),
                        )
                    # single copy: [128, 4 groups, 64] strided into 65-wide slots
                    nc.vector.tensor_copy(
                        v_all[:, tt, :].rearrange("p (g c) -> p g c", g=4)[:, :, 0:HD],
                        vp[:].rearrange("p (g c) -> p g c", g=4),
                    )

            def attention_chunk(pair, c, ps_st, ps_ot, mid_filler=None):
                ot = ps_ot.tile([HD + 1, 2 * TC], F32, name="ot", bufs=1)
                for kt in range(N_TT):
                    if kt == 8 and mid_filler is not None:
                        mid_filler()
                    st = ps_st.tile([P, 2 * TC], F32, name="st", tag="st")
                    nc.tensor.matmul(
                        st[:, 0:TC],
                        k_ro[pair][0:HD, ts(kt, P)],
                        q_ro[pair][0:HD, ts(c, TC)],
                        start=True, stop=True,
                    )
                    nc.tensor.matmul(
                        st[:, TC : 2 * TC],
                        k_ro[pair][HD:P, ts(kt, P)],
                        q_ro[pair][HD:P, ts(c, TC)],
                        start=True, stop=True,
                        tile_position=(64, 0),
                    )
                    ex = expp.tile([P, 2 * TC], BF16, name="ex")
                    nc.scalar.activation(ex[:], st[:], AF.Exp, scale=SCALE)
                    for hd_i in range(2):
                        g = 2 * pair + hd_i
                        nc.tensor.matmul(
                            ot[:, ts(hd_i, TC)],
                            v_all[:, kt, g * (HD + 1) : (g + 1) * (HD + 1)],
                            ex[:, ts(hd_i, TC)],
                            start=(kt == 0), stop=(kt == N_TT - 1),
                        )
                # normalization: copy out psum fast, reciprocal on the sum row,
                # broadcast, two muls; h2 half moves partitions via one DMA
                n65 = nrm.tile([HD + 1, 2 * TC], F32, name="n65")
                nc.vector.tensor_copy(n65[:], ot[:])
                inv = nrm.tile([1, 2 * TC], F32, name="inv")
                nc.vector.reciprocal(inv[:], n65[HD : HD + 1, :])
                bsum = nrm.tile([HD, 2 * TC], F32, name="bsum")
                nc.gpsimd.partition_broadcast(bsum[:], inv[:])
                nc.vector.tensor_mul(
                    o_sb[pair][0:HD, ts(c, TC)], n65[0:HD, 0:TC], bsum[:, 0:TC]
                )
                onrm = nrm.tile([HD, TC], BF16, name="onrm")
                nc.vector.tensor_mul(onrm[:], n65[0:HD, TC : 2 * TC], bsum[:, TC : 2 * TC])
                nc.gpsimd.dma_start(o_sb[pair][HD:P, ts(c, TC)], onrm[:])

            def o_proj_mm(t, fcs, ps_qk):
                ops = []
                for fc in fcs:
                    op = ps_qk.tile(
                        [P, TC], F32, name="op", tag=("qp" if fc % 2 == 0 else "kp"), bufs=1
                    )
                    for pair in range(N_PAIRS):
                        nc.tensor.matmul(
                            op[:],
                            wo_sb[:, pair, ts(fc, P)],
                            o_sb[pair][:, ts(t, TC)],
                            start=(pair == 0), stop=(pair == N_PAIRS - 1),
                        )
                    ops.append((fc, op))
                return ops

            def o_proj_flush(t, ops, last=False):
                for fc, op in ops:
                    # bounce via SBUF (DMA cannot read PSUM)
                    ob = outst.tile([P, TC], F32, name="ob")
                    if last and fc % 2 == 1:
                        nc.scalar.copy(ob[:], op[:])
                    else:
                        nc.vector.tensor_copy(ob[:], op[:])
                    if last:
                        out_eng = (nc.sync, nc.gpsimd, nc.scalar)[fc % 3]
                    else:
                        out_eng = (nc.sync, nc.gpsimd)[fc % 2]
                    out_eng.dma_start(out_d[ts(fc, P), ts(t, TC)], ob[:])

            # ---- emission order drives scheduler priority ----
            with tc.tile_pool(name="ps_qk", bufs=1, space="PSUM") as ps_qk:
                with tc.tile_pool(name="ps_st", bufs=2, space="PSUM") as ps_st:
                    with tc.tile_pool(name="ps_ot", bufs=1, space="PSUM") as ps_ot:
                        # input DMAs: x quarters split over two queues, ahead
                        # of cos/sin which are only needed once rope starts
                        nc.sync.dma_start(wk_sb[:], wk_d[:])
                        nc.scalar.dma_start(xsb[:, 0], xT_d[:, 0])
                        nc.sync.dma_start(xsb[:, 1], xT_d[:, 1])
                        nc.gpsimd.dma_start(cos_sb[:], cos_d[:])
                        nc.gpsimd.dma_start(sin_sb[:], sin_d[:])
                        nc.scalar.dma_start(xsb[:, 2], xT_d[:, 2])
                        nc.sync.dma_start(xsb[:, 3], xT_d[:, 3])
                        nc.sync.dma_start(wq_sb[:], wq_d[:])
                        nc.gpsimd.dma_start(wv_sb[:], wv_d[:])
                        nc.gpsimd.dma_start(wo_sb[:], wo_d[:])

                        # prologue: k first (scores need all of k), then q c0 + v,
                        # so exp can start as early as possible
                        for t in range(N_TC):
                            proj_chunk(k_ro[0], wk_sb, 0, t, "qp" if t % 2 == 0 else "kp", ps_qk)
                        proj_chunk(q_ro[0], wq_sb, 0, 0, "qp", ps_qk)
                        v_tiles(range(0, 4), ps_qk)
                        for t in range(1, N_TC):
                            proj_chunk(q_ro[0], wq_sb, 0, t, "qp" if t % 2 == 0 else "kp", ps_qk)
                        v_tiles(range(4, N_TT), ps_qk)

                        # attention pair 0 with pair-1 projections as PE filler,
                        # one chunk mid-attention and one at the chunk boundary so
                        # psum-bank reuse chains get ~9us of slack
                        p1 = [
                            (k_ro[1], wk_sb), (k_ro[1], wk_sb), (k_ro[1], wk_sb),
                            (k_ro[1], wk_sb), (q_ro[1], wq_sb), (q_ro[1], wq_sb),
                            (q_ro[1], wq_sb), (q_ro[1], wq_sb),
                        ]
                        p1_t = [0, 1, 2, 3, 0, 1, 2, 3]
                        fi = iter(range(8))

                        def filler1():
                            i = next(fi)
                            dst, w = p1[i]
                            proj_chunk(dst, w, 1, p1_t[i], "qp" if i % 2 == 0 else "kp", ps_qk)

                        for c in range(N_TC):
                            attention_chunk(0, c, ps_st, ps_ot, mid_filler=filler1)
                            filler1()

                        # attention pair 1 with o-proj as staggered filler: two
                        # matmuls mid-chunk (psum held), bounce copies deferred
                        # past the norm so the ot-releasing copy keeps DVE priority
                        for c in range(N_TC):
                            if c >= 1:
                                held = {}

                                def mid(t=c - 1, held=held):
                                    held["ops"] = o_proj_mm(t, range(0, 2), ps_qk)

                                attention_chunk(1, c, ps_st, ps_ot, mid_filler=mid)
                                o_proj_flush(c - 1, held["ops"])
                                for fc in range(2, HC, 2):
                                    pair_ops = o_proj_mm(c - 1, range(fc, fc + 2), ps_qk)
                                    o_proj_flush(c - 1, pair_ops)
                            else:
                                attention_chunk(1, c, ps_st, ps_ot)
                        for fc in range(0, HC, 2):
                            pair_ops = o_proj_mm(N_TC - 1, range(fc, fc + 2), ps_qk)
                            o_proj_flush(N_TC - 1, pair_ops, last=True)

    nc.compile()
    return nc, names


_CACHE = {}


def _get_program():
    if "prog" not in _CACHE:
        _CACHE["prog"] = build_program()
    return _CACHE["prog"]


def _rope_tables():
    inv_freq = 1.0 / (BASE ** (np.arange(0, HD, 2, dtype=np.float64) / HD))
    t = np.arange(L, dtype=np.float64)
    freqs = np.outer(t, inv_freq)            # [L, 32]
    emb = np.concatenate((freqs, freqs), -1)  # [L, 64]
    cos = np.cos(emb).T.astype(np.float32)    # [64, L]
    sin = np.sin(emb).T.astype(np.float32)    # [64, L]
    sin_signed = sin.copy()
    sin_signed[: HD // 2] *= -1.0             # rotate_half sign baked in
    cosT = np.ascontiguousarray(np.concatenate([cos, cos], 0))      # [128, L]
    sinT = np.ascontiguousarray(np.concatenate([sin_signed, sin_signed], 0))
    return cosT.astype(ml_dtypes.bfloat16), sinT.astype(ml_dtypes.bfloat16)


def _part_major_w(wT):
    """[1024, E] -> [128, 8, E] with hidden chunk-major partitions."""
    e = wT.shape[1]
    return np.ascontiguousarray(wT.reshape(HC, P, e).transpose(1, 0, 2))


def make_in_maps(names, x, Wq, Wk, Wv, Wo):
    cosT, sinT = _rope_tables()
    bf = ml_dtypes.bfloat16
    in_maps = []
    # x -> [128, quarter, hidden-chunk, 512] partition-major, contiguous per
    # partition per quarter (fast DMA descriptors)
    xTs = []
    for b in range(B):
        xT = x[b].T.astype(bf)                       # [1024, 2048]
        xp = xT.reshape(HC, P, N_TC, TC).transpose(1, 2, 0, 3)  # [128, 4, 8, 512]
        xTs.append(np.ascontiguousarray(xp))
    for core in range(8):
        b = core // 4
        g = core % 4
        es = slice(g * E_LOCAL, (g + 1) * E_LOCAL)
        m = {
            names["in"][0]: xTs[b],
            names["in"][1]: _part_major_w(Wq[es, :].T.astype(bf)),
            names["in"][2]: _part_major_w(Wk[es, :].T.astype(bf)),
            names["in"][3]: _part_major_w(Wv[es, :].T.astype(bf)),
            names["in"][4]: np.ascontiguousarray(
                Wo[:, es].T.astype(bf).reshape(2, P, HIDDEN).transpose(1, 0, 2)
            ),
            names["in"][5]: cosT,
            names["in"][6]: sinT,
        }
        in_maps.append(m)
    return in_maps


def gather_out(names, res):
    out = np.zeros((B, L, HIDDEN), dtype=np.float32)
    for b in range(B):
        acc = np.zeros((HIDDEN, L), dtype=np.float32)
        for g in range(4):
            acc += np.asarray(res.results[b * 4 + g][names["out"]], dtype=np.float32)
        out[b] = acc.T
    return out


def kernel(x, Wq, Wk, Wv, Wo):
    x = np.asarray(x, dtype=np.float32)
    Wq = np.asarray(Wq, dtype=np.float32)
    Wk = np.asarray(Wk, dtype=np.float32)
    Wv = np.asarray(Wv, dtype=np.float32)
    Wo = np.asarray(Wo, dtype=np.float32)

    nc, names = _get_program()
    in_maps = make_in_maps(names, x, Wq, Wk, Wv, Wo)
    res = run_bass_kernel_spmd(nc, in_maps, core_ids=list(range(8)))
    return gather_out(names, res)


# revision 20
# speedup vs baseline: 1.0468x; 1.0171x over previous
"""Trainium2 Bass kernel: 16-head RoPE attention (B=2, L=2048, HIDDEN=1024).

Sharding: 8 cores = 2 batches x 4 head-groups (4 heads per core).
v2: all matmul inputs bf16 (same PE rate as fp32r, half the SBUF/DMA),
x resident in SBUF (loaded once), scalar queue runs exp only, o-proj
DMAs psum->DRAM directly, short normalization chain (reciprocal on one
partition + partition_broadcast), k-proj-first prologue so exp starts
early.
"""

import numpy as np
import ml_dtypes
from contextlib import ExitStack

from concourse import bacc, tile, mybir
from concourse.bass import ts
from concourse.bass_utils import run_bass_kernel_spmd

HIDDEN = 1024
HEADS = 16
HD = 64
L = 2048
B = 2
BASE = 10000.0

P = 128
E_LOCAL = 256          # 4 heads per core
N_PAIRS = 2            # head pairs per core (2 heads on 128 partitions)
HC = HIDDEN // P       # 8 hidden chunks
TC = 512               # token chunk (matmul free dim)
N_TC = L // TC         # 4
N_TT = L // P          # 16 token tiles (for v / k-tiles)
SCALE = 1.0 / 8.0      # 1/sqrt(HD)

F32 = mybir.dt.float32
BF16 = mybir.dt.bfloat16
AF = mybir.ActivationFunctionType


def build_program(debug=False):
    nc = bacc.Bacc(None, target_bir_lowering=False)
    names = {}
    with tile.TileContext(nc) as tc:
        ctx = ExitStack()
        with ctx:
            dram = ctx.enter_context(tc.tile_pool(name="dram", bufs=1, space="DRAM"))
            # host pre-arranges everything partition-major so each DMA is 128
            # large contiguous descriptors (fast DGE) instead of ~1024 small ones
            xT_d = dram.tile([P, N_TC, HC, TC], BF16, kind="ExternalInput", name="xT")
            wq_d = dram.tile([P, HC, E_LOCAL], BF16, kind="ExternalInput", name="wq")
            wk_d = dram.tile([P, HC, E_LOCAL], BF16, kind="ExternalInput", name="wk")
            wv_d = dram.tile([P, HC, E_LOCAL], BF16, kind="ExternalInput", name="wv")
            wo_d = dram.tile([P, 2, HIDDEN], BF16, kind="ExternalInput", name="wo")
            cos_d = dram.tile([P, L], BF16, kind="ExternalInput", name="cosT")
            sin_d = dram.tile([P, L], BF16, kind="ExternalInput", name="sinT")
            out_d = dram.tile([HIDDEN, L], BF16, kind="ExternalOutput", name="outT")
            names["in"] = [t.tensor.name for t in (xT_d, wq_d, wk_d, wv_d, wo_d, cos_d, sin_d)]
            names["out"] = out_d.tensor.name

            # ---------------- persistent SBUF ----------------
            const = ctx.enter_context(tc.tile_pool(name="const", bufs=1))
            wq_sb = const.tile([P, HC, E_LOCAL], BF16)
            wk_sb = const.tile([P, HC, E_LOCAL], BF16)
            wv_sb = const.tile([P, HC, E_LOCAL], BF16)
            wo_sb = const.tile([P, 2, HIDDEN], BF16)
            cos_sb = const.tile([P, L], BF16)
            sin_sb = const.tile([P, L], BF16)
            xsb = const.tile([P, N_TC, HC, TC], BF16)

            # rope'd q and k, feature-major: per pair [128, L] bf16
            qkro = ctx.enter_context(tc.tile_pool(name="qkro", bufs=1))
            q_ro = [qkro.tile([P, L], BF16, name=f"q_ro{p}") for p in range(N_PAIRS)]
            k_ro = [qkro.tile([P, L], BF16, name=f"k_ro{p}") for p in range(N_PAIRS)]
            # v token-major with ones columns: [128 tok, tt, 4*65] bf16
            v_all = qkro.tile([P, N_TT, 4 * (HD + 1)], BF16)
            ones_sb = qkro.tile([P, N_TT], BF16)
            nc.vector.memset(ones_sb[:], 1.0)
            for g in range(4):
                nc.vector.tensor_copy(
                    v_all[:, :, g * (HD + 1) + HD : g * (HD + 1) + HD + 1],
                    ones_sb[:].rearrange("p (a b) -> p a b", b=1),
                )
            # normalized attention output, feature-major per pair [128, L] bf16
            o_sb = [qkro.tile([P, L], BF16, name=f"o_sb{p}") for p in range(N_PAIRS)]

            # ---------------- working pools ----------------
            rope_t = ctx.enter_context(tc.tile_pool(name="rope", bufs=2))
            expp = ctx.enter_context(tc.tile_pool(name="expp", bufs=3))
            nrm = ctx.enter_context(tc.tile_pool(name="nrm", bufs=2))
            outst = ctx.enter_context(tc.tile_pool(name="outst", bufs=2))

            def rope_chunk(dst, ps_tile, t):
                """psum [128, TC] -> dst[:, t*TC:(t+1)*TC] with RoPE applied."""
                raw = rope_t.tile([P, TC], F32, name="raw")
                shuf = rope_t.tile([P, TC], F32, name="shuf")
                t1 = rope_t.tile([P, TC], F32, name="t1")
                t2 = rope_t.tile([P, TC], F32, name="t2")
                # fast psum release: one copy out, everything else reads raw
                nc.vector.tensor_copy(raw[:], ps_tile[:])
                # swap 32-partition halves within each 64-row head block
                nc.gpsimd.dma_start(shuf[0:32, :], raw[32:64, :])
                nc.gpsimd.dma_start(shuf[32:64, :], raw[0:32, :])
                nc.sync.dma_start(shuf[64:96, :], raw[96:128, :])
                nc.sync.dma_start(shuf[96:128, :], raw[64:96, :])
                nc.vector.tensor_mul(t1[:], raw[:], cos_sb[:, ts(t, TC)])
                nc.vector.tensor_mul(t2[:], shuf[:], sin_sb[:, ts(t, TC)])
                nc.vector.tensor_add(dst[:, ts(t, TC)], t1[:], t2[:])

            def proj_chunk(dst_ro, w_sb, pair, t, tag, ps_qk):
                """One [128, TC] projection chunk for q or k + rope."""
                pp = ps_qk.tile([P, TC], F32, name=tag, tag=tag, bufs=1)
                for h in range(HC):
                    nc.tensor.matmul(
                        pp[:], w_sb[:, h, ts(pair, P)], xsb[:, t, h, :],
                        start=(h == 0), stop=(h == HC - 1),
                    )
                rope_chunk(dst_ro, pp, t)

            def v_tiles(tt_list, ps_qk):
                for tt in tt_list:
                    q, s = tt // 4, tt % 4
                    vp = ps_qk.tile(
                        [P, E_LOCAL], F32, name="vp",
                        tag=("qp" if tt % 2 == 0 else "kp"), bufs=1,
                    )
                    for h in range(HC):
                        nc.tensor.matmul(
                            vp[:], xsb[:, q, h, ts(s, P)], wv_sb[:, h, :],
                            start=(h == 0), stop=(h == HC - 1),
                        )
                    # single copy: [128, 4 groups, 64] strided into 65-wide slots
                    nc.vector.tensor_copy(
                        v_all[:, tt, :].rearrange("p (g c) -> p g c", g=4)[:, :, 0:HD],
                        vp[:].rearrange("p (g c) -> p g c", g=4),
                    )

            def attention_chunk(pair, c, ps_st, ps_ot, mid_filler=None):
                ot = ps_ot.tile([HD + 1, 2 * TC], F32, name="ot", bufs=1)
                for kt in range(N_TT):
                    if kt == 8 and mid_filler is not None:
                        mid_filler()
                    st = ps_st.tile([P, 2 * TC], F32, name="st", tag="st")
                    nc.tensor.matmul(
                        st[:, 0:TC],
                        k_ro[pair][0:HD, ts(kt, P)],
                        q_ro[pair][0:HD, ts(c, TC)],
                        start=True, stop=True,
                    )
                    nc.tensor.matmul(
                        st[:, TC : 2 * TC],
                        k_ro[pair][HD:P, ts(kt, P)],
                        q_ro[pair][HD:P, ts(c, TC)],
                        start=True, stop=True,
                        tile_position=(64, 0),
                    )
                    ex = expp.tile([P, 2 * TC], BF16, name="ex")
                    nc.scalar.activation(ex[:], st[:], AF.Exp, scale=SCALE)
                    for hd_i in range(2):
                        g = 2 * pair + hd_i
                        nc.tensor.matmul(
                            ot[:, ts(hd_i, TC)],
                            v_all[:, kt, g * (HD + 1) : (g + 1) * (HD + 1)],
                            ex[:, ts(hd_i, TC)],
                            start=(kt == 0), stop=(kt == N_TT - 1),
                        )
                # normalization: copy out psum fast, reciprocal on the sum row,
                # broadcast, two muls; h2 half moves partitions via one DMA
                n65 = nrm.tile([HD + 1, 2 * TC], F32, name="n65")
                nc.vector.tensor_copy(n65[:], ot[:])
                inv = nrm.tile([1, 2 * TC], F32, name="inv")
                nc.vector.reciprocal(inv[:], n65[HD : HD + 1, :])
                bsum = nrm.tile([HD, 2 * TC], F32, name="bsum")
                nc.gpsimd.partition_broadcast(bsum[:], inv[:])
                nc.vector.tensor_mul(
                    o_sb[pair][0:HD, ts(c, TC)], n65[0:HD, 0:TC], bsum[:, 0:TC]
                )
                onrm = nrm.tile([HD, TC], BF16, name="onrm")
                nc.vector.tensor_mul(onrm[:], n65[0:HD, TC : 2 * TC], bsum[:, TC : 2 * TC])
                nc.gpsimd.dma_start(o_sb[pair][HD:P, ts(c, TC)], onrm[:])

            def o_proj_mm(t, fcs, ps_qk):
                ops = []
                for fc in fcs:
                    op = ps_qk.tile(
                        [P, TC], F32, name="op", tag=("qp" if fc % 2 == 0 else "kp"), bufs=1
                    )
                    for pair in range(N_PAIRS):
                        nc.tensor.matmul(
                            op[:],
                            wo_sb[:, pair, ts(fc, P)],
                            o_sb[pair][:, ts(t, TC)],
                            start=(pair == 0), stop=(pair == N_PAIRS - 1),
                        )
                    ops.append((fc, op))
                return ops

            def o_proj_flush(t, ops, last=False):
                for fc, op in ops:
                    # bounce via SBUF in bf16 (halves output wire bytes)
                    ob = outst.tile([P, TC], BF16, name="ob")
                    if last and fc % 2 == 1:
                        nc.scalar.copy(ob[:], op[:])
                    else:
                        nc.vector.tensor_copy(ob[:], op[:])
                    if last:
                        out_eng = (nc.sync, nc.gpsimd, nc.scalar)[fc % 3]
                    else:
                        out_eng = nc.sync
                    out_eng.dma_start(out_d[ts(fc, P), ts(t, TC)], ob[:])

            # ---- emission order drives scheduler priority ----
            with tc.tile_pool(name="ps_qk", bufs=1, space="PSUM") as ps_qk:
                with tc.tile_pool(name="ps_st", bufs=2, space="PSUM") as ps_st:
                    with tc.tile_pool(name="ps_ot", bufs=1, space="PSUM") as ps_ot:
                        # input DMAs: x quarters split over two queues, ahead
                        # of cos/sin which are only needed once rope starts
                        nc.sync.dma_start(wk_sb[:], wk_d[:])
                        nc.scalar.dma_start(xsb[:, 0], xT_d[:, 0])
                        nc.sync.dma_start(xsb[:, 1], xT_d[:, 1])
                        nc.gpsimd.dma_start(cos_sb[:], cos_d[:])
                        nc.gpsimd.dma_start(sin_sb[:], sin_d[:])
                        nc.scalar.dma_start(xsb[:, 2], xT_d[:, 2])
                        nc.sync.dma_start(xsb[:, 3], xT_d[:, 3])
                        nc.sync.dma_start(wq_sb[:], wq_d[:])
                        nc.gpsimd.dma_start(wv_sb[:], wv_d[:])
                        nc.gpsimd.dma_start(wo_sb[:], wo_d[:])

                        # prologue: k first (scores need all of k), then q c0 + v,
                        # so exp can start as early as possible
                        for t in range(N_TC):
                            proj_chunk(k_ro[0], wk_sb, 0, t, "qp" if t % 2 == 0 else "kp", ps_qk)
                        proj_chunk(q_ro[0], wq_sb, 0, 0, "qp", ps_qk)
                        v_tiles(range(0, 4), ps_qk)
                        for t in range(1, N_TC):
                            proj_chunk(q_ro[0], wq_sb, 0, t, "qp" if t % 2 == 0 else "kp", ps_qk)
                        v_tiles(range(4, N_TT), ps_qk)

                        # attention pair 0 with pair-1 projections as PE filler,
                        # one chunk mid-attention and one at the chunk boundary so
                        # psum-bank reuse chains get ~9us of slack
                        p1 = [
                            (k_ro[1], wk_sb), (k_ro[1], wk_sb), (k_ro[1], wk_sb),
                            (k_ro[1], wk_sb), (q_ro[1], wq_sb), (q_ro[1], wq_sb),
                            (q_ro[1], wq_sb), (q_ro[1], wq_sb),
                        ]
                        p1_t = [0, 1, 2, 3, 0, 1, 2, 3]
                        fi = iter(range(8))

                        def filler1():
                            i = next(fi)
                            dst, w = p1[i]
                            proj_chunk(dst, w, 1, p1_t[i], "qp" if i % 2 == 0 else "kp", ps_qk)

                        for c in range(N_TC):
                            attention_chunk(0, c, ps_st, ps_ot, mid_filler=filler1)
                            filler1()

                        # attention pair 1 with o-proj as staggered filler: two
                        # matmuls mid-chunk (psum held), bounce copies deferred
                        # past the norm so the ot-releasing copy keeps DVE priority
                        for c in range(N_TC):
                            if c >= 1:
                                held = {}

                                def mid(t=c - 1, held=held):
                                    held["ops"] = o_proj_mm(t, range(0, 2), ps_qk)

                                attention_chunk(1, c, ps_st, ps_ot, mid_filler=mid)
                                o_proj_flush(c - 1, held["ops"])
                                for fc in range(2, HC, 2):
                                    pair_ops = o_proj_mm(c - 1, range(fc, fc + 2), ps_qk)
                                    o_proj_flush(c - 1, pair_ops)
                            else:
                                attention_chunk(1, c, ps_st, ps_ot)
                        for fc in range(0, HC, 2):
                            pair_ops = o_proj_mm(N_TC - 1, range(fc, fc + 2), ps_qk)
                            o_proj_flush(N_TC - 1, pair_ops, last=True)

    nc.compile()
    return nc, names


_CACHE = {}


def _get_program():
    if "prog" not in _CACHE:
        _CACHE["prog"] = build_program()
    return _CACHE["prog"]


def _rope_tables():
    inv_freq = 1.0 / (BASE ** (np.arange(0, HD, 2, dtype=np.float64) / HD))
    t = np.arange(L, dtype=np.float64)
    freqs = np.outer(t, inv_freq)            # [L, 32]
    emb = np.concatenate((freqs, freqs), -1)  # [L, 64]
    cos = np.cos(emb).T.astype(np.float32)    # [64, L]
    sin = np.sin(emb).T.astype(np.float32)    # [64, L]
    sin_signed = sin.copy()
    sin_signed[: HD // 2] *= -1.0             # rotate_half sign baked in
    cosT = np.ascontiguousarray(np.concatenate([cos, cos], 0))      # [128, L]
    sinT = np.ascontiguousarray(np.concatenate([sin_signed, sin_signed], 0))
    return cosT.astype(ml_dtypes.bfloat16), sinT.astype(ml_dtypes.bfloat16)


def _part_major_w(wT):
    """[1024, E] -> [128, 8, E] with hidden chunk-major partitions."""
    e = wT.shape[1]
    return np.ascontiguousarray(wT.reshape(HC, P, e).transpose(1, 0, 2))


def make_in_maps(names, x, Wq, Wk, Wv, Wo):
    cosT, sinT = _rope_tables()
    bf = ml_dtypes.bfloat16
    in_maps = []
    # x -> [128, quarter, hidden-chunk, 512] partition-major, contiguous per
    # partition per quarter (fast DMA descriptors)
    xTs = []
    for b in range(B):
        xT = x[b].T.astype(bf)                       # [1024, 2048]
        xp = xT.reshape(HC, P, N_TC, TC).transpose(1, 2, 0, 3)  # [128, 4, 8, 512]
        xTs.append(np.ascontiguousarray(xp))
    for core in range(8):
        b = core // 4
        g = core % 4
        es = slice(g * E_LOCAL, (g + 1) * E_LOCAL)
        m = {
            names["in"][0]: xTs[b],
            names["in"][1]: _part_major_w(Wq[es, :].T.astype(bf)),
            names["in"][2]: _part_major_w(Wk[es, :].T.astype(bf)),
            names["in"][3]: _part_major_w(Wv[es, :].T.astype(bf)),
            names["in"][4]: np.ascontiguousarray(
                Wo[:, es].T.astype(bf).reshape(2, P, HIDDEN).transpose(1, 0, 2)
            ),
            names["in"][5]: cosT,
            names["in"][6]: sinT,
        }
        in_maps.append(m)
    return in_maps


def gather_out(names, res):
    out = np.zeros((B, L, HIDDEN), dtype=np.float32)
    for b in range(B):
        acc = np.zeros((HIDDEN, L), dtype=np.float32)
        for g in range(4):
            acc += np.asarray(res.results[b * 4 + g][names["out"]], dtype=np.float32)
        out[b] = acc.T
    return out


def kernel(x, Wq, Wk, Wv, Wo):
    x = np.asarray(x, dtype=np.float32)
    Wq = np.asarray(Wq, dtype=np.float32)
    Wk = np.asarray(Wk, dtype=np.float32)
    Wv = np.asarray(Wv, dtype=np.float32)
    Wo = np.asarray(Wo, dtype=np.float32)

    nc, names = _get_program()
    in_maps = make_in_maps(names, x, Wq, Wk, Wv, Wo)
    res = run_bass_kernel_spmd(nc, in_maps, core_ids=list(range(8)))
    return gather_out(names, res)
